# revision 30
# baseline (speedup 1.0000x reference)
"""Trainium2 Bass kernel for nn_Direction_Attention_layer (sparse_attention).

Math (S == D == 512):
    uit  = tanh(x @ W + b)                      [B, S, D]
    a    = exp(uit @ U)                         [B, S, D]
    fw_a[b,d] = EPS + sum_{s > d} a[b,s,d]      (mask couples seq idx with feat idx)
    bw_a[b,d] = EPS + sum_{s < d} a[b,s,d]
    xs[b,d]   = sum_s x[b,s,d]
    out  = concat(fw_a * xs, bw_a * xs)         [B, 2D]

Sharding: data-parallel over batch B=64 across 8 cores (8 batches/core);
W/U/b replicated. No collectives.

Precision strategy (validated numerically): both matmuls run in bf16 (PE
streams 2 cols/cycle + fast weight load), while xs and all post-exp
accumulation stay fp32. The tanh/exp/sum structure averages out the bf16
quantization; end-to-end error is ~1.9e-4 fro-relative vs the fp32 reference,
vs ~1.1e-4 for an all-fp32r variant that runs ~3x slower on the PE.

Per-core layout is fully transposed: the host pre-transposes x to [B, D, S]
(staging, like sharding), so tiles load with 2KB-contiguous descriptors, both
matmuls keep the contraction dim on partitions, and the masked sums decompose
into:
  - full 128-col blocks left/right of the diagonal block -> fused into the
    exp activation via accum_out (free-axis sum, no extra passes)
  - the diagonal 128x128 block -> per-f DVE masked multiply + reduce
Both directions are accumulated as sums of positives (no cancellation).
"""

import sys

sys.path.insert(0, "/opt/trn_rl_repo")

import numpy as np

B, S, D = 64, 512, 512
N_CORES = 8
BPC = B // N_CORES  # batches per core
NT = D // 128  # 4 partition tiles
EPS = 1e-7

_NC_CACHE = {}


def _build_nc(repeat: int = 1):
    import concourse.bass as bass
    import concourse.tile as tile
    from concourse import bacc, mybir

    FP32 = mybir.dt.float32
    BF16 = mybir.dt.bfloat16
    AX = mybir.AxisListType
    OP = mybir.AluOpType
    AF = mybir.ActivationFunctionType

    nc = bacc.Bacc("TRN2", target_bir_lowering=False, debug=False, num_devices=N_CORES)

    # xf: host-pre-transposed fp32 x shard [BPC, D, S] (feeds xs reduction)
    # xb: same data in bf16 (feeds matmul 1)
    xf_ext = nc.declare_dram_parameter("xf", [BPC, D, S], FP32, isOutput=False)
    xb_ext = nc.declare_dram_parameter("xb", [BPC, D, S], BF16, isOutput=False)
    w_ext = nc.declare_dram_parameter("w", [D, D], BF16, isOutput=False)
    u_ext = nc.declare_dram_parameter("u", [D, D], BF16, isOutput=False)
    b_ext = nc.declare_dram_parameter("bvec", [D], FP32, isOutput=False)
    fwm_ext = nc.declare_dram_parameter("fwmask", [128, D], FP32, isOutput=False)
    bwm_ext = nc.declare_dram_parameter("bwmask", [128, D], FP32, isOutput=False)
    o_ext = nc.declare_dram_parameter("o", [BPC, 2 * D], FP32, isOutput=True)

    with tile.TileContext(nc) as tc:
        with (
            tc.tile_pool(name="consts", bufs=1) as cpool,
            tc.tile_pool(name="xf", bufs=2) as xf_pool,
            tc.tile_pool(name="xb", bufs=3) as xb_pool,
            tc.tile_pool(name="uitt", bufs=2) as uit_pool,
            tc.tile_pool(name="diag", bufs=2) as diag_pool,
            tc.tile_pool(name="scr", bufs=2) as scr_pool,
            tc.tile_pool(name="masked", bufs=2) as msk_pool,
            tc.tile_pool(name="sums", bufs=2) as sum_pool,
            tc.tile_pool(name="outsb", bufs=2) as out_pool,
            tc.tile_pool(name="ps1", bufs=4, space="PSUM") as ps1_pool,
            tc.tile_pool(name="ps2", bufs=4, space="PSUM") as ps2_pool,
        ):
            # ---- constants ----
            w_t = cpool.tile([128, NT, D], BF16)  # w_t[p,k,e] = W[128k+p, e]
            u_t = cpool.tile([128, NT, D], BF16)
            bias = cpool.tile([128, NT], FP32)  # bias[p,e] = b[128e+p]
            fwm = cpool.tile([128, D], FP32)
            bwm = cpool.tile([128, D], FP32)

            def load_w_head():
                # bias is 2KB and gates the first tanh — land it first
                nc.sync.dma_start(
                    out=bias[:], in_=b_ext.rearrange("(e p) -> p e", p=128)
                )
                # just w[k0, e0-cols]: all the first matmul needs from W
                nc.sync.dma_start(out=w_t[:, 0, 0:128], in_=w_ext[0:128, 0:128])

            def load_w_rest():
                nc.sync.dma_start(out=w_t[:, 0, 128:D], in_=w_ext[0:128, 128:D])
                for k in range(1, NT):
                    nc.sync.dma_start(
                        out=w_t[:, k, :], in_=w_ext[128 * k : 128 * (k + 1), :]
                    )

            def load_consts_late():
                # U per-f slices: mm2(b0, f) can start as soon as slice f lands
                for f in range(NT):
                    nc.sync.dma_start(
                        out=u_t[:, :, 128 * f : 128 * (f + 1)],
                        in_=u_ext[:, 128 * f : 128 * (f + 1)].rearrange(
                            "(k p) f -> p k f", p=128
                        ),
                    )
                nc.sync.dma_start(out=fwm[:], in_=fwm_ext[:])
                nc.sync.dma_start(out=bwm[:], in_=bwm_ext[:])

            def load(b, split=False):
                """xb/xf[p,k,s] = x[b, s, 128k+p]; host pre-transposed, so
                contiguous-descriptor DMAs (bf16 split per-k for batch 0 so
                the first matmul starts earliest)."""
                xb = xb_pool.tile([128, NT, S], BF16, tag="xb")
                if split:
                    for k in range(NT):
                        nc.sync.dma_start(
                            out=xb[:, k, :],
                            in_=xb_ext[b, 128 * k : 128 * (k + 1), :],
                        )
                else:
                    nc.sync.dma_start(
                        out=xb[:], in_=xb_ext[b].rearrange("(k p) s -> p k s", p=128)
                    )
                xf = xf_pool.tile([128, NT, S], FP32, tag="xf")
                nc.sync.dma_start(
                    out=xf[:], in_=xf_ext[b].rearrange("(k p) s -> p k s", p=128)
                )
                return xf, xb

            def mm1(xb):
                """uitT[p,e,s] = tanh((x @ W)^T + b), written as bf16."""
                uitt = uit_pool.tile([128, NT, S], BF16, tag="uitt")
                for e in range(NT):
                    ps1 = ps1_pool.tile([128, S], FP32, tag="ps1")
                    for k in range(NT):
                        nc.tensor.matmul(
                            ps1[:],
                            lhsT=w_t[:, k, 128 * e : 128 * (e + 1)],
                            rhs=xb[:, k, :],
                            start=(k == 0),
                            stop=(k == NT - 1),
                        )
                    nc.scalar.activation(
                        uitt[:, e, :], ps1[:], AF.Tanh, bias=bias[:, e : e + 1]
                    )
                return uitt

            def mm2_post(b, xf, uitt):
                """a^T = exp((uit @ U)^T); masked sums; write output row b.

                Accumulators live in [128, 8] tiles, cols 0-3 = fw per f_tile,
                cols 4-7 = bw, matching the output layout directly."""
                # xs2[p,k] = xs2[p,4+k] = sum_s x[b, s, 128k+p] (exact fp32)
                xs2 = sum_pool.tile([128, 2 * NT], FP32, tag="xs2")
                for k in range(NT):
                    nc.vector.tensor_reduce(
                        out=xs2[:, k : k + 1], in_=xf[:, k, :], axis=AX.X, op=OP.add
                    )
                nc.vector.tensor_copy(xs2[:, NT : 2 * NT], xs2[:, 0:NT])

                pb = sum_pool.tile([128, 2 * NT], FP32, tag="pb")
                nc.vector.memset(pb[:], 0.0)

                diag = diag_pool.tile([128, D], FP32, tag="diag")
                scr = scr_pool.tile([128, S], FP32, tag="scr")
                mfw = msk_pool.tile([128, NT, 128], FP32, tag="mfw")
                mbw = msk_pool.tile([128, NT, 128], FP32, tag="mbw")
                dd = sum_pool.tile([128, 2 * NT], FP32, tag="dd")
                for f in range(NT):
                    ps2 = ps2_pool.tile([128, S], FP32, tag="ps2")
                    for e in range(NT):
                        nc.tensor.matmul(
                            ps2[:],
                            lhsT=u_t[:, e, 128 * f : 128 * (f + 1)],
                            rhs=uitt[:, e, :],
                            start=(e == 0),
                            stop=(e == NT - 1),
                        )
                    lo, hi = 128 * f, 128 * (f + 1)
                    # diagonal 128x128 block first so DVE can start early;
                    # masked sums per-f: dd[p,f] = sum_j diag_f[p,j]*(j>p),
                    # dd[p,4+f] with (j<p)
                    nc.scalar.activation(diag[:, lo:hi], ps2[:, lo:hi], AF.Exp)
                    nc.vector.tensor_tensor(
                        out=mfw[:, f, :],
                        in0=diag[:, lo:hi],
                        in1=fwm[:, lo:hi],
                        op=OP.mult,
                    )
                    nc.vector.tensor_reduce(
                        out=dd[:, f : f + 1], in_=mfw[:, f, :], axis=AX.X, op=OP.add
                    )
                    nc.vector.tensor_tensor(
                        out=mbw[:, f, :],
                        in0=diag[:, lo:hi],
                        in1=bwm[:, lo:hi],
                        op=OP.mult,
                    )
                    nc.vector.tensor_reduce(
                        out=dd[:, NT + f : NT + f + 1],
                        in_=mbw[:, f, :],
                        axis=AX.X,
                        op=OP.add,
                    )
                    # strictly-below-diag cols: all s < d for this tile -> bw
                    if f > 0:
                        nc.scalar.activation(
                            scr[:, 0:lo],
                            ps2[:, 0:lo],
                            AF.Exp,
                            accum_out=pb[:, NT + f : NT + f + 1],
                        )
                    # strictly-above-diag cols: all s > d -> fw
                    if f < NT - 1:
                        nc.scalar.activation(
                            scr[:, hi:S],
                            ps2[:, hi:S],
                            AF.Exp,
                            accum_out=pb[:, f : f + 1],
                        )

                # out[b, half*512 + 128f + p] = (partial + diag + EPS) * xs
                osb = out_pool.tile([128, 2 * NT], FP32, tag="osb")
                nc.vector.tensor_tensor(out=osb[:], in0=pb[:], in1=dd[:], op=OP.add)
                nc.vector.tensor_scalar_add(osb[:], osb[:], EPS)
                nc.vector.tensor_tensor(out=osb[:], in0=osb[:], in1=xs2[:], op=OP.mult)
                nc.sync.dma_start(
                    out=o_ext[b].rearrange("(c p) -> p c", p=128), in_=osb[:]
                )

            # software-pipelined schedule: mm1(b) ahead of mm2(b-1) so the PE
            # never waits on the tanh chain
            def body(first_iter):
                state = []  # (b, xf, uitt)
                for b in range(BPC + 1):
                    if b < BPC:
                        if b == 0 and first_iter:
                            load_w_head()
                        xf, xb = load(b, split=(b == 0 and first_iter))
                        if b == 0 and first_iter:
                            load_w_rest()
                            load_consts_late()
                        uitt = mm1(xb)
                        state.append((b, xf, uitt))
                    if b >= 1:
                        pb_, pxf, puitt = state[b - 1]
                        mm2_post(pb_, pxf, puitt)

            if repeat == 1:
                body(True)
            else:
                # benchmarking mode: repeat the whole computation on-device in
                # a hardware loop so per-iteration time is measurable above
                # host/axon dispatch noise
                load_w_head()
                load_w_rest()
                load_consts_late()
                with tc.For_i(0, repeat, 1):
                    body(False)

    nc.finalize()
    return nc


def _make_mask_inputs():
    j = np.arange(128, dtype=np.int64)
    blk_fw = (j[None, :] > j[:, None]).astype(np.float32)  # j > p
    blk_bw = (j[None, :] < j[:, None]).astype(np.float32)  # j < p
    return np.tile(blk_fw, (1, NT)), np.tile(blk_bw, (1, NT))


def _make_in_maps(x, W, U, b):
    import ml_dtypes

    # host staging: shard, pre-transpose to [B, D, S], cast matmul copies to bf16
    xt = np.ascontiguousarray(
        np.asarray(x, dtype=np.float32).transpose(0, 2, 1)
    )
    xb = xt.astype(ml_dtypes.bfloat16)
    wb = np.asarray(W, dtype=np.float32).astype(ml_dtypes.bfloat16)
    ub = np.asarray(U, dtype=np.float32).astype(ml_dtypes.bfloat16)
    bf = np.ascontiguousarray(b, dtype=np.float32)
    fwmask, bwmask = _make_mask_inputs()
    return [
        {
            "xf": xt[c * BPC : (c + 1) * BPC],
            "xb": xb[c * BPC : (c + 1) * BPC],
            "w": wb,
            "u": ub,
            "bvec": bf,
            "fwmask": fwmask,
            "bwmask": bwmask,
        }
        for c in range(N_CORES)
    ]


def kernel(x, W, U, b):
    from concourse.bass_utils import run_bass_kernel_spmd

    x = np.asarray(x)
    assert x.shape == (B, S, D)
    key = "nc"
    if key not in _NC_CACHE:
        _NC_CACHE[key] = _build_nc()
    nc = _NC_CACHE[key]

    in_maps = _make_in_maps(x, np.asarray(W), np.asarray(U), np.asarray(b))
    res = run_bass_kernel_spmd(nc, in_maps, list(range(N_CORES)))
    out = np.concatenate([res.results[c]["o"] for c in range(N_CORES)], axis=0)
    return out.astype(np.float32)


# revision 33
# speedup vs baseline: 1.0621x; 1.0621x over previous
"""Trainium2 Bass kernel for nn_Direction_Attention_layer (sparse_attention).

Math (S == D == 512):
    uit  = tanh(x @ W + b)                      [B, S, D]
    a    = exp(uit @ U)                         [B, S, D]
    fw_a[b,d] = EPS + sum_{s > d} a[b,s,d]      (mask couples seq idx with feat idx)
    bw_a[b,d] = EPS + sum_{s < d} a[b,s,d]
    xs[b,d]   = sum_s x[b,s,d]
    out  = concat(fw_a * xs, bw_a * xs)         [B, 2D]

Sharding: data-parallel over batch B=64 across 8 cores (8 batches/core);
W/U/b replicated. No collectives.

Precision strategy (validated numerically): both matmuls run in bf16 (PE
streams 2 cols/cycle + fast weight load), while xs and all post-exp
accumulation stay fp32. The tanh/exp/sum structure averages out the bf16
quantization; end-to-end error is ~1.9e-4 fro-relative vs the fp32 reference,
vs ~1.1e-4 for an all-fp32r variant that runs ~3x slower on the PE.

Per-core layout is fully transposed: the host pre-transposes x to [B, D, S]
(staging, like sharding), so tiles load with 2KB-contiguous descriptors, both
matmuls keep the contraction dim on partitions, and the masked sums decompose
into:
  - full 128-col blocks left/right of the diagonal block -> fused into the
    exp activation via accum_out (free-axis sum, no extra passes)
  - the diagonal 128x128 block -> per-f DVE masked multiply + reduce
Both directions are accumulated as sums of positives (no cancellation).
"""

import sys

sys.path.insert(0, "/opt/trn_rl_repo")

import numpy as np

B, S, D = 64, 512, 512
N_CORES = 8
BPC = B // N_CORES  # batches per core
NT = D // 128  # 4 partition tiles
EPS = 1e-7

_NC_CACHE = {}


def _build_nc(repeat: int = 1):
    import concourse.bass as bass
    import concourse.tile as tile
    from concourse import bacc, mybir

    FP32 = mybir.dt.float32
    BF16 = mybir.dt.bfloat16
    AX = mybir.AxisListType
    OP = mybir.AluOpType
    AF = mybir.ActivationFunctionType

    nc = bacc.Bacc("TRN2", target_bir_lowering=False, debug=False, num_devices=N_CORES)

    # xf: host-pre-transposed fp32 x shard [BPC, D, S] (feeds xs reduction)
    # xb: same data in bf16 (feeds matmul 1)
    xf_ext = nc.declare_dram_parameter("xf", [BPC, D, S], FP32, isOutput=False)
    xb_ext = nc.declare_dram_parameter("xb", [BPC, D, S], BF16, isOutput=False)
    w_ext = nc.declare_dram_parameter("w", [D, D], BF16, isOutput=False)
    u_ext = nc.declare_dram_parameter("u", [D, D], BF16, isOutput=False)
    b_ext = nc.declare_dram_parameter("bvec", [D], FP32, isOutput=False)
    fwm_ext = nc.declare_dram_parameter("fwmask", [128, D], FP32, isOutput=False)
    bwm_ext = nc.declare_dram_parameter("bwmask", [128, D], FP32, isOutput=False)
    o_ext = nc.declare_dram_parameter("o", [BPC, 2 * D], FP32, isOutput=True)

    with tile.TileContext(nc) as tc:
        with (
            tc.tile_pool(name="consts", bufs=1) as cpool,
            tc.tile_pool(name="xf", bufs=2) as xf_pool,
            tc.tile_pool(name="xb", bufs=3) as xb_pool,
            tc.tile_pool(name="uitt", bufs=2) as uit_pool,
            tc.tile_pool(name="diag", bufs=2) as diag_pool,
            tc.tile_pool(name="scr", bufs=2) as scr_pool,
            tc.tile_pool(name="masked", bufs=2) as msk_pool,
            tc.tile_pool(name="sums", bufs=2) as sum_pool,
            tc.tile_pool(name="outsb", bufs=2) as out_pool,
            tc.tile_pool(name="ps1", bufs=4, space="PSUM") as ps1_pool,
            tc.tile_pool(name="ps2", bufs=4, space="PSUM") as ps2_pool,
        ):
            # ---- constants ----
            w_t = cpool.tile([128, NT, D], BF16)  # w_t[p,k,e] = W[128k+p, e]
            u_t = cpool.tile([128, NT, D], BF16)
            bias = cpool.tile([128, NT], FP32)  # bias[p,e] = b[128e+p]
            fwm = cpool.tile([128, D], FP32)
            bwm = cpool.tile([128, D], FP32)

            def load_w_head():
                # bias is 2KB and gates the first tanh — land it first
                nc.sync.dma_start(
                    out=bias[:], in_=b_ext.rearrange("(e p) -> p e", p=128)
                )
                # just w[k0, e0-cols]: all the first matmul needs from W
                nc.sync.dma_start(out=w_t[:, 0, 0:128], in_=w_ext[0:128, 0:128])

            def load_w_rest():
                nc.sync.dma_start(out=w_t[:, 0, 128:D], in_=w_ext[0:128, 128:D])
                for k in range(1, NT):
                    nc.sync.dma_start(
                        out=w_t[:, k, :], in_=w_ext[128 * k : 128 * (k + 1), :]
                    )

            def load_consts_late():
                # U per-f slices: mm2(b0, f) can start as soon as slice f lands
                for f in range(NT):
                    nc.sync.dma_start(
                        out=u_t[:, :, 128 * f : 128 * (f + 1)],
                        in_=u_ext[:, 128 * f : 128 * (f + 1)].rearrange(
                            "(k p) f -> p k f", p=128
                        ),
                    )
                nc.sync.dma_start(out=fwm[:], in_=fwm_ext[:])
                nc.sync.dma_start(out=bwm[:], in_=bwm_ext[:])

            def load(b, split=False):
                """xb/xf[p,k,s] = x[b, s, 128k+p]; host pre-transposed, so
                contiguous-descriptor DMAs (bf16 split per-k for batch 0 so
                the first matmul starts earliest)."""
                xb = xb_pool.tile([128, NT, S], BF16, tag="xb")
                if split:
                    for k in range(NT):
                        nc.sync.dma_start(
                            out=xb[:, k, :],
                            in_=xb_ext[b, 128 * k : 128 * (k + 1), :],
                        )
                else:
                    nc.sync.dma_start(
                        out=xb[:], in_=xb_ext[b].rearrange("(k p) s -> p k s", p=128)
                    )
                xf = xf_pool.tile([128, NT, S], FP32, tag="xf")
                nc.sync.dma_start(
                    out=xf[:], in_=xf_ext[b].rearrange("(k p) s -> p k s", p=128)
                )
                return xf, xb

            def mm1(xb):
                """uitT[p,e,s] = tanh((x @ W)^T + b), written as bf16."""
                uitt = uit_pool.tile([128, NT, S], BF16, tag="uitt")
                for e in range(NT):
                    ps1 = ps1_pool.tile([128, S], FP32, tag="ps1")
                    for k in range(NT):
                        nc.tensor.matmul(
                            ps1[:],
                            lhsT=w_t[:, k, 128 * e : 128 * (e + 1)],
                            rhs=xb[:, k, :],
                            start=(k == 0),
                            stop=(k == NT - 1),
                        )
                    nc.scalar.activation(
                        uitt[:, e, :], ps1[:], AF.Tanh, bias=bias[:, e : e + 1]
                    )
                return uitt

            def mm2_post(b, xf, uitt):
                """a^T = exp((uit @ U)^T); masked sums; write output row b.

                Accumulators live in [128, 8] tiles, cols 0-3 = fw per f_tile,
                cols 4-7 = bw, matching the output layout directly."""
                # xs2[p,k] = xs2[p,4+k] = sum_s x[b, s, 128k+p] (exact fp32)
                xs2 = sum_pool.tile([128, 2 * NT], FP32, tag="xs2")
                nc.vector.tensor_reduce(
                    out=xs2[:, 0:NT], in_=xf[:], axis=AX.X, op=OP.add
                )
                nc.vector.tensor_copy(xs2[:, NT : 2 * NT], xs2[:, 0:NT])

                pb = sum_pool.tile([128, 2 * NT], FP32, tag="pb")
                nc.vector.memset(pb[:], 0.0)

                diag = diag_pool.tile([128, D], FP32, tag="diag")
                scr = scr_pool.tile([128, S], FP32, tag="scr")
                mfw = msk_pool.tile([128, D], FP32, tag="mfw")
                mbw = msk_pool.tile([128, D], FP32, tag="mbw")
                dd = sum_pool.tile([128, 2 * NT], FP32, tag="dd")
                for f in range(NT):
                    ps2 = ps2_pool.tile([128, S], FP32, tag="ps2")
                    for e in range(NT):
                        nc.tensor.matmul(
                            ps2[:],
                            lhsT=u_t[:, e, 128 * f : 128 * (f + 1)],
                            rhs=uitt[:, e, :],
                            start=(e == 0),
                            stop=(e == NT - 1),
                        )
                    lo, hi = 128 * f, 128 * (f + 1)
                    # diagonal 128x128 block, masked on DVE after the f-loop
                    nc.scalar.activation(diag[:, lo:hi], ps2[:, lo:hi], AF.Exp)
                    # strictly-below-diag cols: all s < d for this tile -> bw
                    if f > 0:
                        nc.scalar.activation(
                            scr[:, 0:lo],
                            ps2[:, 0:lo],
                            AF.Exp,
                            accum_out=pb[:, NT + f : NT + f + 1],
                        )
                    # strictly-above-diag cols: all s > d -> fw
                    if f < NT - 1:
                        nc.scalar.activation(
                            scr[:, hi:S],
                            ps2[:, hi:S],
                            AF.Exp,
                            accum_out=pb[:, f : f + 1],
                        )

                # masked diagonal sums, one big op per direction:
                # dd[p,f] = sum_j diag[p, 128f+j]*(j>p); dd[p,4+f] with (j<p)
                nc.vector.tensor_tensor(out=mfw[:], in0=diag[:], in1=fwm[:], op=OP.mult)
                nc.vector.tensor_reduce(
                    out=dd[:, 0:NT],
                    in_=mfw.rearrange("p (f j) -> p f j", j=128),
                    axis=AX.X,
                    op=OP.add,
                )
                nc.vector.tensor_tensor(out=mbw[:], in0=diag[:], in1=bwm[:], op=OP.mult)
                nc.vector.tensor_reduce(
                    out=dd[:, NT : 2 * NT],
                    in_=mbw.rearrange("p (f j) -> p f j", j=128),
                    axis=AX.X,
                    op=OP.add,
                )

                # out[b, half*512 + 128f + p] = (partial + diag + EPS) * xs
                osb = out_pool.tile([128, 2 * NT], FP32, tag="osb")
                nc.vector.tensor_tensor(out=osb[:], in0=pb[:], in1=dd[:], op=OP.add)
                nc.vector.tensor_scalar_add(osb[:], osb[:], EPS)
                nc.vector.tensor_tensor(out=osb[:], in0=osb[:], in1=xs2[:], op=OP.mult)
                nc.sync.dma_start(
                    out=o_ext[b].rearrange("(c p) -> p c", p=128), in_=osb[:]
                )

            # software-pipelined schedule: mm1(b) ahead of mm2(b-1) so the PE
            # never waits on the tanh chain
            def body(first_iter):
                state = []  # (b, xf, uitt)
                for b in range(BPC + 1):
                    if b < BPC:
                        if b == 0 and first_iter:
                            load_w_head()
                        xf, xb = load(b, split=(b == 0 and first_iter))
                        if b == 0 and first_iter:
                            load_w_rest()
                            load_consts_late()
                        uitt = mm1(xb)
                        state.append((b, xf, uitt))
                    if b >= 1:
                        pb_, pxf, puitt = state[b - 1]
                        mm2_post(pb_, pxf, puitt)

            if repeat == 1:
                body(True)
            else:
                # benchmarking mode: repeat the whole computation on-device in
                # a hardware loop so per-iteration time is measurable above
                # host/axon dispatch noise
                load_w_head()
                load_w_rest()
                load_consts_late()
                with tc.For_i(0, repeat, 1):
                    body(False)

    nc.finalize()
    return nc


def _make_mask_inputs():
    j = np.arange(128, dtype=np.int64)
    blk_fw = (j[None, :] > j[:, None]).astype(np.float32)  # j > p
    blk_bw = (j[None, :] < j[:, None]).astype(np.float32)  # j < p
    return np.tile(blk_fw, (1, NT)), np.tile(blk_bw, (1, NT))


def _make_in_maps(x, W, U, b):
    import ml_dtypes

    # host staging: shard, pre-transpose to [B, D, S], cast matmul copies to bf16
    xt = np.ascontiguousarray(
        np.asarray(x, dtype=np.float32).transpose(0, 2, 1)
    )
    xb = xt.astype(ml_dtypes.bfloat16)
    wb = np.asarray(W, dtype=np.float32).astype(ml_dtypes.bfloat16)
    ub = np.asarray(U, dtype=np.float32).astype(ml_dtypes.bfloat16)
    bf = np.ascontiguousarray(b, dtype=np.float32)
    fwmask, bwmask = _make_mask_inputs()
    return [
        {
            "xf": xt[c * BPC : (c + 1) * BPC],
            "xb": xb[c * BPC : (c + 1) * BPC],
            "w": wb,
            "u": ub,
            "bvec": bf,
            "fwmask": fwmask,
            "bwmask": bwmask,
        }
        for c in range(N_CORES)
    ]


def kernel(x, W, U, b):
    from concourse.bass_utils import run_bass_kernel_spmd

    x = np.asarray(x)
    assert x.shape == (B, S, D)
    key = "nc"
    if key not in _NC_CACHE:
        _NC_CACHE[key] = _build_nc()
    nc = _NC_CACHE[key]

    in_maps = _make_in_maps(x, np.asarray(W), np.asarray(U), np.asarray(b))
    res = run_bass_kernel_spmd(nc, in_maps, list(range(N_CORES)))
    out = np.concatenate([res.results[c]["o"] for c in range(N_CORES)], axis=0)
    return out.astype(np.float32)


# revision 38
# speedup vs baseline: 1.0979x; 1.0337x over previous
"""Trainium2 Bass kernel for nn_Direction_Attention_layer (sparse_attention).

Math (S == D == 512):
    uit  = tanh(x @ W + b)                      [B, S, D]
    a    = exp(uit @ U)                         [B, S, D]
    fw_a[b,d] = EPS + sum_{s > d} a[b,s,d]      (mask couples seq idx with feat idx)
    bw_a[b,d] = EPS + sum_{s < d} a[b,s,d]
    xs[b,d]   = sum_s x[b,s,d]
    out  = concat(fw_a * xs, bw_a * xs)         [B, 2D]

Sharding: data-parallel over batch B=64 across 8 cores (8 batches/core);
W/U/b replicated. No collectives.

Precision strategy (validated numerically): both matmuls run in bf16 (PE
streams 2 cols/cycle + fast weight load), while xs and all post-exp
accumulation stay fp32. The tanh/exp/sum structure averages out the bf16
quantization; end-to-end error is ~1.9e-4 fro-relative vs the fp32 reference,
vs ~1.1e-4 for an all-fp32r variant that runs ~3x slower on the PE.

Per-core layout is fully transposed: the host pre-transposes x to [B, D, S]
(staging, like sharding), so tiles load with 2KB-contiguous descriptors, both
matmuls keep the contraction dim on partitions, and the masked sums decompose
into:
  - full 128-col blocks left/right of the diagonal block -> fused into the
    exp activation via accum_out (free-axis sum, no extra passes)
  - the diagonal 128x128 block -> per-f DVE masked multiply + reduce
Both directions are accumulated as sums of positives (no cancellation).
"""

import sys

sys.path.insert(0, "/opt/trn_rl_repo")

import numpy as np

B, S, D = 64, 512, 512
N_CORES = 8
BPC = B // N_CORES  # batches per core
NT = D // 128  # 4 partition tiles
EPS = 1e-7

_NC_CACHE = {}


def _build_nc(repeat: int = 1):
    import concourse.bass as bass
    import concourse.tile as tile
    from concourse import bacc, mybir

    FP32 = mybir.dt.float32
    BF16 = mybir.dt.bfloat16
    AX = mybir.AxisListType
    OP = mybir.AluOpType
    AF = mybir.ActivationFunctionType

    nc = bacc.Bacc("TRN2", target_bir_lowering=False, debug=False, num_devices=N_CORES)

    # host-pre-transposed x shard [BPC, D, S], split as bf16 hi + bf16 residual:
    # hi feeds matmul 1; hi+lo reconstruct x to ~2^-17 for the xs reduction.
    # Half the DMA traffic of shipping fp32 x + a bf16 matmul copy.
    xh_ext = nc.declare_dram_parameter("xh", [BPC, D, S], BF16, isOutput=False)
    xl_ext = nc.declare_dram_parameter("xl", [BPC, D, S], BF16, isOutput=False)
    w_ext = nc.declare_dram_parameter("w", [D, D], BF16, isOutput=False)
    u_ext = nc.declare_dram_parameter("u", [D, D], BF16, isOutput=False)
    b_ext = nc.declare_dram_parameter("bvec", [D], FP32, isOutput=False)
    fwm_ext = nc.declare_dram_parameter("fwmask", [128, D], FP32, isOutput=False)
    bwm_ext = nc.declare_dram_parameter("bwmask", [128, D], FP32, isOutput=False)
    o_ext = nc.declare_dram_parameter("o", [BPC, 2 * D], FP32, isOutput=True)

    with tile.TileContext(nc) as tc:
        with (
            tc.tile_pool(name="consts", bufs=1) as cpool,
            tc.tile_pool(name="xf", bufs=2) as xf_pool,
            tc.tile_pool(name="xb", bufs=3) as xb_pool,
            tc.tile_pool(name="uitt", bufs=2) as uit_pool,
            tc.tile_pool(name="diag", bufs=2) as diag_pool,
            tc.tile_pool(name="scr", bufs=2) as scr_pool,
            tc.tile_pool(name="masked", bufs=2) as msk_pool,
            tc.tile_pool(name="sums", bufs=2) as sum_pool,
            tc.tile_pool(name="outsb", bufs=2) as out_pool,
            tc.tile_pool(name="ps1", bufs=4, space="PSUM") as ps1_pool,
            tc.tile_pool(name="ps2", bufs=4, space="PSUM") as ps2_pool,
        ):
            # ---- constants ----
            w_t = cpool.tile([128, NT, D], BF16)  # w_t[p,k,e] = W[128k+p, e]
            u_t = cpool.tile([128, NT, D], BF16)
            bias = cpool.tile([128, NT], FP32)  # bias[p,e] = b[128e+p]
            fwm = cpool.tile([128, D], FP32)
            bwm = cpool.tile([128, D], FP32)

            def load_w_head():
                # bias is 2KB and gates the first tanh — land it first
                nc.sync.dma_start(
                    out=bias[:], in_=b_ext.rearrange("(e p) -> p e", p=128)
                )
                # just w[k0, e0-cols]: all the first matmul needs from W
                nc.sync.dma_start(out=w_t[:, 0, 0:128], in_=w_ext[0:128, 0:128])

            def load_w_rest():
                nc.sync.dma_start(out=w_t[:, 0, 128:D], in_=w_ext[0:128, 128:D])
                for k in range(1, NT):
                    nc.sync.dma_start(
                        out=w_t[:, k, :], in_=w_ext[128 * k : 128 * (k + 1), :]
                    )

            def load_consts_late():
                # U per-f slices: mm2(b0, f) can start as soon as slice f lands
                for f in range(NT):
                    nc.sync.dma_start(
                        out=u_t[:, :, 128 * f : 128 * (f + 1)],
                        in_=u_ext[:, 128 * f : 128 * (f + 1)].rearrange(
                            "(k p) f -> p k f", p=128
                        ),
                    )
                nc.sync.dma_start(out=fwm[:], in_=fwm_ext[:])
                nc.sync.dma_start(out=bwm[:], in_=bwm_ext[:])

            def load(b, split=False):
                """xh/xl[p,k,s] = hi/lo of x[b, s, 128k+p]; host
                pre-transposed, so contiguous-descriptor DMAs (hi split per-k
                for batch 0 so the first matmul starts earliest)."""
                xh = xb_pool.tile([128, NT, S], BF16, tag="xh")
                if split:
                    for k in range(NT):
                        nc.sync.dma_start(
                            out=xh[:, k, :],
                            in_=xh_ext[b, 128 * k : 128 * (k + 1), :],
                        )
                else:
                    nc.sync.dma_start(
                        out=xh[:], in_=xh_ext[b].rearrange("(k p) s -> p k s", p=128)
                    )
                xl = xf_pool.tile([128, NT, S], BF16, tag="xl")
                nc.sync.dma_start(
                    out=xl[:], in_=xl_ext[b].rearrange("(k p) s -> p k s", p=128)
                )
                return xl, xh

            def mm1(xb):
                """uitT[p,e,s] = tanh((x @ W)^T + b), written as bf16."""
                uitt = uit_pool.tile([128, NT, S], BF16, tag="uitt")
                for e in range(NT):
                    ps1 = ps1_pool.tile([128, S], FP32, tag="ps1")
                    for k in range(NT):
                        nc.tensor.matmul(
                            ps1[:],
                            lhsT=w_t[:, k, 128 * e : 128 * (e + 1)],
                            rhs=xb[:, k, :],
                            start=(k == 0),
                            stop=(k == NT - 1),
                        )
                    nc.scalar.activation(
                        uitt[:, e, :], ps1[:], AF.Tanh, bias=bias[:, e : e + 1]
                    )
                return uitt

            def mm2_post(b, xh, xl, uitt):
                """a^T = exp((uit @ U)^T); masked sums; write output row b.

                Accumulators live in [128, 8] tiles, cols 0-3 = fw per f_tile,
                cols 4-7 = bw, matching the output layout directly."""
                # xs2[p,k] = xs2[p,4+k] = sum_s x[b, s, 128k+p]
                # fp32 accumulation over bf16 hi + lo halves (~2^-17 accurate)
                xs2 = sum_pool.tile([128, 2 * NT], FP32, tag="xs2")
                xsl = sum_pool.tile([128, NT], FP32, tag="xsl")
                nc.vector.tensor_reduce(
                    out=xs2[:, 0:NT], in_=xh[:], axis=AX.X, op=OP.add
                )
                nc.vector.tensor_reduce(out=xsl[:], in_=xl[:], axis=AX.X, op=OP.add)
                nc.vector.tensor_tensor(
                    out=xs2[:, 0:NT], in0=xs2[:, 0:NT], in1=xsl[:], op=OP.add
                )
                nc.vector.tensor_copy(xs2[:, NT : 2 * NT], xs2[:, 0:NT])

                pb = sum_pool.tile([128, 2 * NT], FP32, tag="pb")
                nc.vector.memset(pb[:], 0.0)

                diag = diag_pool.tile([128, D], FP32, tag="diag")
                scr = scr_pool.tile([128, S], FP32, tag="scr")
                mfw = msk_pool.tile([128, D], FP32, tag="mfw")
                mbw = msk_pool.tile([128, D], FP32, tag="mbw")
                dd = sum_pool.tile([128, 2 * NT], FP32, tag="dd")
                for f in range(NT):
                    ps2 = ps2_pool.tile([128, S], FP32, tag="ps2")
                    for e in range(NT):
                        nc.tensor.matmul(
                            ps2[:],
                            lhsT=u_t[:, e, 128 * f : 128 * (f + 1)],
                            rhs=uitt[:, e, :],
                            start=(e == 0),
                            stop=(e == NT - 1),
                        )
                    lo, hi = 128 * f, 128 * (f + 1)
                    # diagonal 128x128 block, masked on DVE after the f-loop
                    nc.scalar.activation(diag[:, lo:hi], ps2[:, lo:hi], AF.Exp)
                    # strictly-below-diag cols: all s < d for this tile -> bw
                    if f > 0:
                        nc.scalar.activation(
                            scr[:, 0:lo],
                            ps2[:, 0:lo],
                            AF.Exp,
                            accum_out=pb[:, NT + f : NT + f + 1],
                        )
                    # strictly-above-diag cols: all s > d -> fw
                    if f < NT - 1:
                        nc.scalar.activation(
                            scr[:, hi:S],
                            ps2[:, hi:S],
                            AF.Exp,
                            accum_out=pb[:, f : f + 1],
                        )

                # masked diagonal sums, one big op per direction:
                # dd[p,f] = sum_j diag[p, 128f+j]*(j>p); dd[p,4+f] with (j<p)
                nc.vector.tensor_tensor(out=mfw[:], in0=diag[:], in1=fwm[:], op=OP.mult)
                nc.vector.tensor_reduce(
                    out=dd[:, 0:NT],
                    in_=mfw.rearrange("p (f j) -> p f j", j=128),
                    axis=AX.X,
                    op=OP.add,
                )
                nc.vector.tensor_tensor(out=mbw[:], in0=diag[:], in1=bwm[:], op=OP.mult)
                nc.vector.tensor_reduce(
                    out=dd[:, NT : 2 * NT],
                    in_=mbw.rearrange("p (f j) -> p f j", j=128),
                    axis=AX.X,
                    op=OP.add,
                )

                # out[b, half*512 + 128f + p] = (partial + diag + EPS) * xs
                osb = out_pool.tile([128, 2 * NT], FP32, tag="osb")
                nc.vector.tensor_tensor(out=osb[:], in0=pb[:], in1=dd[:], op=OP.add)
                nc.vector.tensor_scalar_add(osb[:], osb[:], EPS)
                nc.vector.tensor_tensor(out=osb[:], in0=osb[:], in1=xs2[:], op=OP.mult)
                nc.sync.dma_start(
                    out=o_ext[b].rearrange("(c p) -> p c", p=128), in_=osb[:]
                )

            # software-pipelined schedule: mm1(b) ahead of mm2(b-1) so the PE
            # never waits on the tanh chain
            def body(first_iter):
                state = []  # (b, xh, xl, uitt)
                for b in range(BPC + 1):
                    if b < BPC:
                        if b == 0 and first_iter:
                            load_w_head()
                        xl, xh = load(b, split=(b == 0 and first_iter))
                        if b == 0 and first_iter:
                            load_w_rest()
                            load_consts_late()
                        uitt = mm1(xh)
                        state.append((b, xh, xl, uitt))
                    if b >= 1:
                        pb_, pxh, pxl, puitt = state[b - 1]
                        mm2_post(pb_, pxh, pxl, puitt)

            if repeat == 1:
                body(True)
            else:
                # benchmarking mode: repeat the whole computation on-device in
                # a hardware loop so per-iteration time is measurable above
                # host/axon dispatch noise
                load_w_head()
                load_w_rest()
                load_consts_late()
                with tc.For_i(0, repeat, 1):
                    body(False)

    nc.finalize()
    return nc


def _make_mask_inputs():
    j = np.arange(128, dtype=np.int64)
    blk_fw = (j[None, :] > j[:, None]).astype(np.float32)  # j > p
    blk_bw = (j[None, :] < j[:, None]).astype(np.float32)  # j < p
    return np.tile(blk_fw, (1, NT)), np.tile(blk_bw, (1, NT))


def _make_in_maps(x, W, U, b):
    import ml_dtypes

    # host staging: shard, pre-transpose to [B, D, S], split into bf16 hi+lo
    xt = np.ascontiguousarray(
        np.asarray(x, dtype=np.float32).transpose(0, 2, 1)
    )
    xh = xt.astype(ml_dtypes.bfloat16)
    xl = (xt - xh.astype(np.float32)).astype(ml_dtypes.bfloat16)
    wb = np.asarray(W, dtype=np.float32).astype(ml_dtypes.bfloat16)
    ub = np.asarray(U, dtype=np.float32).astype(ml_dtypes.bfloat16)
    bf = np.ascontiguousarray(b, dtype=np.float32)
    fwmask, bwmask = _make_mask_inputs()
    return [
        {
            "xh": xh[c * BPC : (c + 1) * BPC],
            "xl": xl[c * BPC : (c + 1) * BPC],
            "w": wb,
            "u": ub,
            "bvec": bf,
            "fwmask": fwmask,
            "bwmask": bwmask,
        }
        for c in range(N_CORES)
    ]


def kernel(x, W, U, b):
    from concourse.bass_utils import run_bass_kernel_spmd

    x = np.asarray(x)
    assert x.shape == (B, S, D)
    key = "nc"
    if key not in _NC_CACHE:
        _NC_CACHE[key] = _build_nc()
    nc = _NC_CACHE[key]

    in_maps = _make_in_maps(x, np.asarray(W), np.asarray(U), np.asarray(b))
    res = run_bass_kernel_spmd(nc, in_maps, list(range(N_CORES)))
    out = np.concatenate([res.results[c]["o"] for c in range(N_CORES)], axis=0)
    return out.astype(np.float32)
